# revision 85
# baseline (speedup 1.0000x reference)
"""Causal self-attention (B=2, T=2048, C=1024, H=16) on 8 Trainium2 NeuronCores.

Sharding: tensor-parallel over heads — each core owns 2 heads for BOTH
batches.  The 8 (batch, 512-token q-chunk) units are processed in order;
after each PAIR of chunks completes, a small (256 KB) AllToAll immediately
redistributes that pair's head-sharded attention outputs as 128-token
full-channel slivers (core r gets sliver r of the pair).  The last two
chunks instead get their own half-size collectives (64-token slivers) so
the final, exposed collective is as small and early as possible.  Each
core runs the output projection for its slivers as PE filler between
attention groups — all collectives and nearly all projection work hide
under later attention; only the last chunk's collective (~7us) plus one
half-sliver projection (~3us) sit on the critical tail.

Hard-won scheduling facts baked into this file:
- collective_compute BLOCKS the issuing gpsimd queue until the collective
  completes, so gpsimd carries collectives and the (chunk-end-only)
  norm partition_broadcasts, and nothing else.
- Input-DMA descriptor generation (DIRECT2D) is paced by ring drain and
  holds its queue hostage for the whole load phase (~30us), so ALL input
  DMAs go through sync, leaving scalar free for exp from ~12us.
- A warm-up AllToAll during the load phase absorbs the inter-core launch
  skew (tens of us) and the CC-core dispatch warm-up (~10us) while the
  cores are DMA-bound anyway; later collectives then start within ~1us.
- The PE clock ramps with sustained use (~2GHz effective when dense,
  1.2GHz after a gap): junk matmuls bridge the final collective's flight
  so the exposed projection runs at full clock.
- reciprocal_approx_* must read SBUF (PSUM input returns garbage), and a
  DVE tensor_tensor cannot read two PSUM operands.
- fp8 (DoubleRow) kqv is 2x faster but random-sign dot products keep the
  full ~3-4% fp8 element error in the output: rel err 3.3e-2 > the 2e-2
  gate.  Everything stays bf16 with fp32 PSUM accumulation (3.9e-3).

Kernel layout choices:
- x is shipped pre-transposed ([KC, 128, cols] tiles; b0's first q-column
  block separately so the kqv projection starts after ~1.3MB has landed).
- kqv bias is folded into the PSUM->SBUF evacuation (DVE tensor_scalar).
- Scores are computed transposed, sT[k, q] = kT_blk.T @ qT, two heads per
  entry back-to-back (disjoint PE row-groups run concurrently), so
  softmax's denominator folds into the AV matmul as an extra ones-column
  of V (lhsT = [v | ones] -> row 64 of yT accumulates sum_k exp).
- Consecutive unmasked score blocks are PAIRED into a 2-bank PSUM tile so
  each ACT exp call covers 1024 columns (halves the per-instruction ACT
  bubble).
- AV matmuls lag their exp by TWO groups so the in-order PE queue never
  waits on ACT.
- Causal masking: upper-triangle-only score blocks are never computed;
  diagonal blocks multiply with a precomputed tril mask on the DVE.
- Division by the denominator: reciprocal_approx_fast (DVE) +
  partition_broadcast (GPSIMD) + tensor_mul (DVE).
- Projections fold their bias via a ones-row (K=1 chunk).
- Emission interleaves b0's remaining kqv + b1's kqv/transposes 1:1 with
  b0's attention groups to keep the TensorEngine dense.
"""

import hashlib
import numpy as np
import ml_dtypes

B, T, C, H = 2, 2048, 1024, 16
HD = C // H            # 64
NCORES = 8
TQ = 512               # q-chunk width
NJ = T // 128          # 16 k-blocks
NQ = T // TQ           # 4 q-chunks
KC = C // 128          # 8 contraction chunks
NCH = 8                # chunks = B * NQ
NG = NCH // 2          # collective groups (chunk pairs)
SLV = 128              # sliver width (tokens) per core per group

bfloat16 = ml_dtypes.bfloat16


def _chunk_bq(c):
    return c // NQ, c % NQ


# ---------------------------------------------------------------- schedules
def _make_schedule(att_mask):
    """Per q-chunk list of (j, n_off, n_len, masked).

    masked is None (no mask), 'tril' (apply causal tril to slab cols 0:128),
    or an int index into the general mask table.
    """
    m = np.asarray(att_mask).reshape(T, T)
    tril = np.tril(np.ones((T, T), m.dtype))
    if np.array_equal(m, tril):
        sched = []
        for Q in range(NQ):
            ent = [(j, 0, TQ, None) for j in range(4 * Q)]
            for j in range(4 * Q, 4 * Q + 4):
                n_off = 128 * (j - 4 * Q)
                ent.append((j, n_off, TQ - n_off, "tril"))
            sched.append(ent)
        return sched, None

    masks = []
    mask_ids = {}
    sched = []
    for Q in range(NQ):
        ent = []
        for j in range(NJ):
            blk = m[Q * TQ:(Q + 1) * TQ, j * 128:(j + 1) * 128].T  # [128k,512q]
            if not blk.any():
                continue
            if blk.all():
                ent.append((j, 0, TQ, None))
                continue
            key = blk.tobytes()
            if key not in mask_ids:
                mask_ids[key] = len(masks)
                masks.append(blk.astype(np.float32))
            ent.append((j, 0, TQ, mask_ids[key]))
        sched.append(ent)
    masks = np.stack(masks) if masks else None
    return sched, masks


def _group_entries(ents):
    """Group consecutive full-width unmasked entries into pairs."""
    groups = []
    i = 0
    while i < len(ents):
        j, n_off, n_len, mid = ents[i]
        if (mid is None and n_len == TQ and i + 1 < len(ents)
                and ents[i + 1][3] is None and ents[i + 1][2] == TQ):
            groups.append((ents[i], ents[i + 1]))
            i += 2
        else:
            groups.append((ents[i],))
            i += 1
    return groups


def _sched_key(sched, masks):
    h = hashlib.sha256(repr(sched).encode())
    if masks is not None:
        h.update(masks.tobytes())
    return h.hexdigest()


# ---------------------------------------------------------------- builder
_BUILD_CACHE = {}


def _build(sched, masks):
    from concourse import bacc, tile, mybir
    from concourse.masks import make_identity

    BF16, F32 = mybir.dt.bfloat16, mybir.dt.float32
    n_masks = 0 if masks is None else masks.shape[0]

    nc = bacc.Bacc("TRN2", target_bir_lowering=False, debug=False,
                   num_devices=NCORES)

    # -------- I/O ----------------------------------------------------------
    # xT0 is shipped in two column-halves (kqv for b0's first columns can
    # start after ~2MB); xT1 and the weights ship as few large DMAs to
    # minimize descriptor-generation occupancy on the sync queue.
    # (fp8 was tried for the kqv GEMM and is 2x faster, but random-sign
    # dot products keep the full ~3-4% fp8 element error in the output:
    # rel err 3.3e-2 > the 2e-2 gate.  bf16 it stays.)
    xT0_d = nc.dram_tensor("xT0", [KC, 128, TQ], BF16,
                           kind="ExternalInput")
    xT0b_d = nc.dram_tensor("xT0b", [KC, 128, TQ], BF16,
                            kind="ExternalInput")
    xT0c_d = nc.dram_tensor("xT0c", [KC, 128, 2 * TQ], BF16,
                            kind="ExternalInput")
    xT1_d = nc.dram_tensor("xT1", [KC, 128, T], BF16, kind="ExternalInput")
    wk_d = nc.dram_tensor("wk", [128, KC * 6 * HD], BF16,
                          kind="ExternalInput")
    bk_d = nc.dram_tensor("bk", [128, 3], F32, kind="ExternalInput")
    wp_d = nc.dram_tensor("wp", [128, KC * C], BF16, kind="ExternalInput")
    bp_d = nc.dram_tensor("bp", [1, C], BF16, kind="ExternalInput")
    if n_masks:
        mk_d = nc.dram_tensor("mk", [n_masks * 128, TQ], BF16,
                              kind="ExternalInput")
    out_d = nc.dram_tensor("out", [NG, SLV, C], BF16, kind="ExternalOutput")

    VW = 2 * HD + 2     # v_ext tile width: [vA | onesA | vB | onesB] = 130

    with tile.TileContext(nc) as tc:
        with tc.tile_pool(name="big", bufs=1) as big, \
             tc.tile_pool(name="work", bufs=1) as work, \
             tc.tile_pool(name="pmm", bufs=2, space="PSUM") as pmm, \
             tc.tile_pool(name="pqk", bufs=2, space="PSUM") as pqk, \
             tc.tile_pool(name="pyt", bufs=2, space="PSUM") as pyt, \
             tc.tile_pool(name="dram", bufs=1, space="DRAM") as dram:

            # ---- persistent SBUF tensors ----------------------------------
            wk_all = big.tile([128, KC * 6 * HD], BF16, name="wk_all",
                              tag="wk_all")
            wk = [wk_all[:, k * 6 * HD:(k + 1) * 6 * HD] for k in range(KC)]
            bkp = big.tile([128, 3], F32, name="bkp", tag="bkp")
            xT = [[big.tile([128, T], BF16, name=f"xT{b}_{k}",
                            tag=f"xT{b}_{k}") for k in range(KC)]
                  for b in range(B)]
            ones_r = big.tile([1, T], BF16, name="ones_r", tag="ones_r")
            wp_all = big.tile([128, KC * C], BF16, name="wp_all",
                              tag="wp_all")
            wp = [wp_all[:, k * C:(k + 1) * C] for k in range(KC)]
            bp = big.tile([1, C], BF16, name="bp", tag="bp")
            ident = big.tile([128, 128], BF16, name="ident", tag="ident")
            ones2d = big.tile([128, 128], BF16, name="ones2d", tag="ones2d")
            trilm = big.tile([128, 128], BF16, name="trilm", tag="trilm")

            # all input DMAs go through the sync queue: descriptor
            # generation (DIRECT2D) is paced by ring drain, and on sync it
            # blocks nothing — scalar stays free for exp, gpsimd for
            # collectives, vector for evacuations.
            nc.vector.memset(ones2d[:, :], 1.0)
            # tril mask (1 on/below diagonal): replaces per-entry gpsimd
            # affine_select with a DVE multiply, keeping gpsimd collective-only
            nc.gpsimd.affine_select(
                out=trilm[:, :], in_=ones2d[:, :],
                compare_op=mybir.AluOpType.is_ge,
                fill=0.0, base=0, pattern=[[1, 128]], channel_multiplier=-1)

            def _dma(out, in_):
                nc.sync.dma_start(out=out, in_=in_)

            _dma(wk_all[:, :], wk_d.ap())
            _dma(bkp[:, :], bk_d.ap())
            for k in range(KC):
                _dma(xT[0][k][:, 0:TQ], xT0_d.ap()[k, :, :])

            # warm-up collective, staged as soon as bkp lands: absorbs the
            # CC-core dispatch warm-up (~10us) and the inter-core launch
            # skew while the cores are still DMA-bound, so the real
            # collectives later start fast and finish at predictable times.
            # Output is never read.  It blocks the gpsimd queue while it
            # flies, which is why gpsimd carries no other work.
            sync_in = dram.tile([NCORES * 128, 1], F32, name="sync_in",
                                tag="sync_in")
            sync_out = dram.tile([NCORES * 128, 1], F32, name="sync_out",
                                 tag="sync_out")
            for r in range(NCORES):
                _dma(sync_in[r * 128:(r + 1) * 128, :], bkp[:, 0:1])
            nc.gpsimd.collective_compute(
                "AllToAll", mybir.AluOpType.bypass,
                replica_groups=[list(range(NCORES))],
                ins=[sync_in.opt()], outs=[sync_out.opt()])

            for k in range(KC):
                _dma(xT[0][k][:, TQ:2 * TQ], xT0b_d.ap()[k, :, :])
            for k in range(KC):
                _dma(xT[0][k][:, 2 * TQ:T], xT0c_d.ap()[k, :, :])
            for k in range(KC):
                _dma(xT[1][k][:, :], xT1_d.ap()[k, :, :])
            # wp/bp are needed only from ~70us on (first sliver projection)
            _dma(wp_all[:, :], wp_d.ap())
            _dma(bp[:, :], bp_d.ap())

            if n_masks:
                mks = big.tile([128, n_masks * TQ], BF16, name="mks",
                               tag="mks")
                for i in range(n_masks):
                    nc.sync.dma_start(out=mks[:, i * TQ:(i + 1) * TQ],
                                      in_=mk_d.ap()[i * 128:(i + 1) * 128, :])

            # per-batch attention tensors
            kT = [big.tile([128, T], BF16, name=f"kT{b}", tag=f"kT{b}")
                  for b in range(B)]
            qT = [big.tile([128, T], BF16, name=f"qT{b}", tag=f"qT{b}")
                  for b in range(B)]
            vT = [big.tile([128, T], BF16, name=f"vT{b}", tag=f"vT{b}")
                  for b in range(B)]
            vx = [big.tile([128, NJ * VW], BF16, name=f"vx{b}", tag=f"vx{b}")
                  for b in range(B)]
            yT = [big.tile([128, T], BF16, name=f"yT{b}", tag=f"yT{b}")
                  for b in range(B)]

            # collective buffers: one in/out pair per chunk-pair group for
            # chunks 0-5 (128-token slivers); chunks 6 and 7 each get their
            # own half-size collective (64-token slivers) so the final,
            # exposed collective is as small and early as possible
            a2a_in = [dram.tile([NCORES * 128, SLV], BF16,
                                name=f"a2a_in{g}", tag=f"a2a_in{g}")
                      for g in range(3)]
            a2a_out = [dram.tile([NCORES * 128, SLV], BF16,
                                 name=f"a2a_out{g}", tag=f"a2a_out{g}")
                       for g in range(3)]
            yg = [big.tile([128, C], BF16, name=f"yg{g}", tag=f"yg{g}")
                  for g in range(3)]
            a2a_in_h = [dram.tile([NCORES * 128, SLV // 2], BF16,
                                  name=f"a2a_inh{u}", tag=f"a2a_inh{u}")
                        for u in range(2)]
            a2a_out_h = [dram.tile([NCORES * 128, SLV // 2], BF16,
                                   name=f"a2a_outh{u}", tag=f"a2a_outh{u}")
                         for u in range(2)]
            ygh = [big.tile([128, C // 2], BF16, name=f"ygh{u}",
                            tag=f"ygh{u}") for u in range(2)]

            # mid-kernel staging DMAs go on sync (scalar must stay free for
            # exp); only the final pair's tail DMAs split across sync+scalar
            _tail = [nc.sync, nc.sync]

            dst = {0: kT, 1: qT, 2: vT}

            def kqv_steps(b, mns):
                # kqvT[m-tile] = wk[:,m].T @ xT; bias folded into evacuation
                for m, n in mns:
                    ps = pmm.tile([128, TQ], F32, name="kqv_ps", tag="mm",
                                  bufs=2)
                    for k in range(KC):
                        nc.tensor.matmul(
                            ps[:, :],
                            wk[k][:, m * 128:(m + 1) * 128],
                            xT[b][k][:, n * TQ:(n + 1) * TQ],
                            start=(k == 0), stop=(k == KC - 1))
                    nc.vector.tensor_scalar_add(
                        dst[m][b][:, n * TQ:(n + 1) * TQ], ps[:, :],
                        bkp[:, m:m + 1])
                    yield

            def transpose_steps(b):
                vx_v = vx[b].rearrange("p (t c) -> p t c", t=NJ)
                for t in range(NJ):
                    tr = pmm.tile([128, 128], BF16, name="tr_ps", tag="mm",
                                  bufs=2)
                    nc.tensor.transpose(tr[:, :],
                                        vT[b][:, t * 128:(t + 1) * 128],
                                        ident[:, :])
                    o = vx_v[:, t, :].rearrange("p (u c) -> p u c", u=2)
                    nc.vector.tensor_copy(
                        o[:, :, 0:HD], tr.rearrange("p (u c) -> p u c", u=2))
                    if t % 4 == 3:
                        yield

            def stage_chunk(c):
                # stage chunk c's slivers into its collective input and
                # fire the collective + yg gather once the input is full.
                # Staging and gather are single strided DMAs (one DIRECT2D
                # issue each instead of 4-8 serialized ones).
                b, Q = _chunk_bq(c)
                if c < 6:
                    g, half = c // 2, c % 2
                    for r4 in range(4):
                        r = 4 * half + r4
                        _tail[r4 % 2].dma_start(
                            out=a2a_in[g][r * 128:(r + 1) * 128, :],
                            in_=yT[b][:, Q * TQ + r4 * SLV:
                                      Q * TQ + (r4 + 1) * SLV])
                    if half == 1:
                        nc.gpsimd.collective_compute(
                            "AllToAll", mybir.AluOpType.bypass,
                            replica_groups=[list(range(NCORES))],
                            ins=[a2a_in[g].opt()], outs=[a2a_out[g].opt()])
                        for k in range(KC):
                            _tail[k % 2].dma_start(
                                out=yg[g][:, k * 128:(k + 1) * 128],
                                in_=a2a_out[g][k * 128:(k + 1) * 128, :])
                else:
                    # per-chunk half-size collective, 64-token slivers
                    u = c - 6
                    HS = SLV // 2
                    se = [nc.sync, nc.scalar] if u == 1 else _tail
                    for r in range(NCORES):
                        se[r % 2].dma_start(
                            out=a2a_in_h[u][r * 128:(r + 1) * 128, :],
                            in_=yT[b][:, Q * TQ + r * HS:
                                      Q * TQ + (r + 1) * HS])
                    nc.gpsimd.collective_compute(
                        "AllToAll", mybir.AluOpType.bypass,
                        replica_groups=[list(range(NCORES))],
                        ins=[a2a_in_h[u].opt()], outs=[a2a_out_h[u].opt()])
                    # gather split over sync+scalar: 8 serialized DIRECT2Ds
                    # on one queue would stagger the yg arrival by ~5us
                    ge = [nc.sync, nc.scalar]
                    for k in range(KC):
                        ge[k % 2].dma_start(
                            out=ygh[u][:, k * HS:(k + 1) * HS],
                            in_=a2a_out_h[u][k * 128:(k + 1) * 128, :])

            def proj_unit(g, nch):
                # project sliver g, output-half nch: [128tok, 512out]
                ps = pmm.tile([128, TQ], F32, name="proj_ps", tag="mm",
                              bufs=2)
                nc.tensor.matmul(
                    ps[:, :], ones_r[0:1, 0:128],
                    bp[0:1, nch * TQ:(nch + 1) * TQ],
                    start=True, stop=False)
                for k in range(KC):
                    nc.tensor.matmul(
                        ps[:, :], yg[g][:, k * 128:(k + 1) * 128],
                        wp[k][:, nch * TQ:(nch + 1) * TQ],
                        start=False, stop=(k == KC - 1))
                osb = work.tile([128, TQ], BF16, name="osb", tag="osb",
                                bufs=3)
                nc.vector.tensor_copy(osb[:, :], ps[:, :])
                _tail[nch].dma_start(
                    out=out_d.ap()[g, :, nch * TQ:(nch + 1) * TQ],
                    in_=osb[:, :])

            def proj_unit_h_bias(nch):
                # open a final-projection psum group with its bias row —
                # no dependency on the last collective, so it runs early
                HS = SLV // 2
                ps = pmm.tile([HS, TQ], F32, name="proj_psh", tag="mm",
                              bufs=2)
                nc.tensor.matmul(
                    ps[:, :], ones_r[0:1, 0:HS],
                    bp[0:1, nch * TQ:(nch + 1) * TQ],
                    start=True, stop=False)
                return ps

            def proj_unit_h(u, nch, ps=None):
                # project half-sliver u (64 tokens, chunk 6+u), half nch;
                # lands in out_d region 3 rows u*64:(u+1)*64
                HS = SLV // 2
                if ps is None:
                    ps = proj_unit_h_bias(nch)
                for k in range(KC):
                    nc.tensor.matmul(
                        ps[:, :], ygh[u][:, k * HS:(k + 1) * HS],
                        wp[k][:, nch * TQ:(nch + 1) * TQ],
                        start=False, stop=(k == KC - 1))
                osb = work.tile([HS, TQ], BF16, name="osbh", tag="osb",
                                bufs=3)
                nc.vector.tensor_copy(osb[:, :], ps[:, :])
                oe = [nc.sync, nc.scalar] if u == 1 else _tail
                oe[nch].dma_start(
                    out=out_d.ap()[NG - 1, u * HS:(u + 1) * HS,
                                   nch * TQ:(nch + 1) * TQ],
                    in_=osb[:, :])

            jps = [None]

            def junk_mm(w=TQ, src=None, pool=None):
                # filler matmul: keeps the PE activity monitor from
                # re-throttling the clock during ACT-bound stretches.
                # pool=pqk routes the junk psum away from pmm when the
                # final projections' psum groups are being held open there.
                if pool is None:
                    jp = pmm.tile([128, TQ], F32, name="junk_ps", tag="mm",
                                  bufs=2)
                else:
                    jp = pool.tile([128, 2 * TQ], F32, name="junk_psq",
                                   tag="qk", bufs=2)
                s = wp_all if src is None else src
                nc.tensor.matmul(jp[:, 0:w], s[:, 0:128],
                                 s[:, 0:w], start=True, stop=True,
                                 skip_group_check=True)
                jps[0] = jp[:, 0:w]

            def attn_steps(c):
                b, Q = _chunk_bq(c)
                ents = sched[Q]
                if not ents:
                    for t in range(2):
                        nc.vector.memset(
                            yT[b][HD * t:HD * (t + 1), Q * TQ:(Q + 1) * TQ],
                            0.0)
                    stage_chunk(c)
                    return
                groups = _group_entries(ents)
                yps = [pyt.tile([HD + 1, TQ], F32, name=f"y_ps{t}", tag="yt",
                                bufs=2) for t in range(2)]
                n_av = {0: 0, 1: 0}   # AV matmuls emitted so far per head
                tot = sum(2 if (mid == "tril" and n_len > 128) else 1
                          for (j, n_off, n_len, mid) in ents)
                total_av = {0: tot, 1: tot}

                def emit_avs(avs):
                    # avs: list of (t, j, src, o_off, o_len)
                    for t, j, src, o_off, o_len in avs:
                        first = n_av[t] == 0
                        last = n_av[t] == total_av[t] - 1
                        nc.tensor.matmul(
                            yps[t][:, o_off:o_off + o_len],
                            vx[b][:, j * VW + t * (HD + 1):
                                  j * VW + (t + 1) * (HD + 1)],
                            src, start=first, stop=last,
                            skip_group_check=True)
                        n_av[t] += 1

                # Per group: both heads' QK matmuls back-to-back (they hit
                # disjoint PE row-groups and can run concurrently), then one
                # exp per head over the group's whole width, then the
                # group-before-last's AV matmuls (two-group lag so the
                # TensorE never waits on ACT; lag-3 was tried and is worse —
                # it delays yps completion and every chunk's tail).
                pending = []
                pending2 = []
                for grp in groups:
                    gw = sum(e[2] for e in grp)     # group column width
                    cur = []
                    sps = []
                    for t in range(2):
                        sp = pqk.tile([128, 2 * TQ], F32, name="s_ps",
                                      tag="qk", bufs=2)
                        o = 0
                        for (j, n_off, n_len, mid) in grp:
                            nc.tensor.matmul(
                                sp[:, o:o + n_len],
                                kT[b][HD * t:HD * (t + 1),
                                      j * 128:(j + 1) * 128],
                                qT[b][HD * t:HD * (t + 1),
                                      Q * TQ + n_off:(Q + 1) * TQ],
                                start=True, stop=True, skip_group_check=True)
                            o += n_len
                        sps.append(sp)
                    for t in range(2):
                        sp = sps[t]
                        slab = work.tile([128, 2 * TQ], BF16, name="slab",
                                         tag="slab", bufs=8)
                        nc.scalar.activation(
                            slab[:, 0:gw], sp[:, 0:gw],
                            mybir.ActivationFunctionType.Exp, scale=0.125)
                        o = 0
                        for (j, n_off, n_len, mid) in grp:
                            if mid == "tril":
                                slab2 = work.tile([128, 128], BF16,
                                                  name="slab2", tag="slab2",
                                                  bufs=8)
                                nc.vector.tensor_mul(
                                    slab2[:, :], slab[:, o:o + 128],
                                    trilm[:, :])
                                cur.append((t, j, slab2[:, :], n_off, 128))
                                if n_len > 128:
                                    cur.append((t, j,
                                                slab[:, o + 128:o + n_len],
                                                n_off + 128, n_len - 128))
                            elif mid is not None:
                                slab2 = work.tile([128, TQ], BF16,
                                                  name="slab2m", tag="slab2m",
                                                  bufs=4)
                                nc.vector.tensor_mul(
                                    slab2[:, 0:n_len], slab[:, o:o + n_len],
                                    mks[:, mid * TQ:mid * TQ + n_len])
                                cur.append((t, j, slab2[:, 0:n_len],
                                            n_off, n_len))
                            else:
                                cur.append((t, j, slab[:, o:o + n_len],
                                            n_off, n_len))
                            o += n_len
                    emit_avs(pending2)
                    pending2 = pending
                    pending = cur
                    yield
                emit_avs(pending2)
                emit_avs(pending)
                # normalize: yT /= denominator (row HD of y psum).
                # (den evacuation must stay off the ACT queue: ACT is
                # in-order, and a copy gated on the chunk's last AV would
                # stall the next chunk's exps behind it.)
                for t in range(2):
                    den0 = work.tile([1, TQ], F32, name="den0", tag="den0",
                                     bufs=4)
                    nc.vector.tensor_copy(den0[:, :], yps[t][HD:HD + 1, :])
                    den = work.tile([1, TQ], F32, name="den", tag="den",
                                    bufs=4)
                    nc.vector.reciprocal_approx_fast(den[:, :], den0[:, :])
                    bc = work.tile([HD, TQ], F32, name="bc", tag="bc", bufs=4)
                    nc.gpsimd.partition_broadcast(bc[:, :], den[:, :])
                    nc.vector.tensor_mul(
                        yT[b][HD * t:HD * (t + 1), Q * TQ:(Q + 1) * TQ],
                        yps[t][0:HD, :], bc[:, :])
                stage_chunk(c)

            # ---- interleaved emission -------------------------------------
            # a few warm-up matmuls on the weights (first DMA to land)
            # start the PE clock ramp before x arrives
            for _ in range(4):
                junk_mm(src=wk_all)
            # k/q projections for b0's first column-chunk only, so chunk-0
            # attention (QK/exp) starts ~10us in; the rest of the kqv work
            # streams through the attention interleave below.
            for _ in kqv_steps(0, [(0, 0), (1, 0)]):
                pass
            make_identity(nc, ident[:, :])
            nc.vector.memset(ones_r[:, :], 1.0)
            for b in range(B):
                vx_v = vx[b].rearrange("p (t c) -> p t c", t=NJ)
                nc.vector.memset(vx_v[:, :, HD::HD + 1], 1.0)

            def v_stream(b):
                ts = transpose_steps(b)   # yields after blocks 4n..4n+3
                kv = kqv_steps(b, [(2, n) for n in range(NQ)])
                for n in range(NQ):
                    next(kv)
                    next(ts)
                    yield

            def chain_steps(*gens):
                for g_ in gens:
                    yield from g_

            # b0: attention chunks 0-3 (28 groups) interleaved 1:1 with the
            # remaining projection work (26 steps).  Emission order
            # guarantees every tensor's writer precedes its first reader:
            # v-stream block n lands before chunk n's AVs, kT/qT column
            # chunk n before chunk n's QKs.
            s1 = chain_steps(*[attn_steps(c) for c in range(0, 4)])
            vstream = v_stream(0)
            kq_rest = kqv_steps(0, [(0, 1), (1, 1), (0, 2), (1, 2),
                                    (0, 3), (1, 3)])
            b1_all = chain_steps(
                kqv_steps(1, [(m, n) for m in range(3) for n in range(NQ)]),
                transpose_steps(1))

            # interleave order: v1, kq(0,1), kq(1,1), v2, kq(0,2), kq(1,2),
            # v3, kq(0,3), kq(1,3), v4, then the 16 b1 steps
            def b0_filler():
                yield next(vstream)
                yield next(kq_rest)
                yield next(kq_rest)
                yield next(vstream)
                yield next(kq_rest)
                yield next(kq_rest)
                yield next(vstream)
                yield next(kq_rest)
                yield next(kq_rest)
                yield next(vstream)
                yield from b1_all

            s2 = b0_filler()
            s2_live = True
            while True:
                if s2_live:
                    s2_live = next(s2, StopIteration) is not StopIteration
                if next(s1, StopIteration) is StopIteration:
                    break
            while s2_live:
                s2_live = next(s2, StopIteration) is not StopIteration

            # b1: attention chunks 4-7 (PE-bound: no junk filler here);
            # sliver projections feed in once their collective has safely
            # landed.  g1's collective can end late (peer skew), so its
            # projections wait until chunk 7.
            feed = {(4, 0): (0, 0), (4, 2): (0, 1),
                    (7, 3): (1, 0), (7, 5): (1, 1),
                    (7, 7): (2, 0), (7, 9): (2, 1)}
            for c in range(4, NCH):
                gi = 0
                for _ in attn_steps(c):
                    if (c, gi) in feed:
                        proj_unit(*feed.pop((c, gi)))
                    gi += 1
                for (cc, gg), u in list(feed.items()):
                    if cc == c and gg >= gi:
                        proj_unit(*feed.pop((cc, gg)))

            # tail: chunk 6's half-sliver projection is real filler while
            # chunk 7's collective flies; junk keeps the clock up for the
            # rest of the flight, then the final projection lands
            proj_unit_h(0, 0)
            proj_unit_h(0, 1)
            # open the final projections' psum groups (bias rows) now —
            # they have no collective dependency and run at full clock;
            # the junk bridge moves to the idle score-psum pool so it
            # cannot WAR-block on the open groups
            ps3 = [proj_unit_h_bias(nch) for nch in range(2)]
            for _ in range(28):
                junk_mm(w=256, pool=pqk)
            for _ in range(30):
                junk_mm(pool=pqk)
            if jps[0] is not None:
                # drain the last junk psum into a region proj_unit_h(1, 0)
                # overwrites right after (emission order => write order)
                josb = work.tile([128, TQ], BF16, name="josb", tag="osb",
                                 bufs=3)
                nc.vector.tensor_copy(josb[:, :], jps[0][:, :])
                nc.scalar.dma_start(
                    out=out_d.ap()[NG - 1, SLV // 2:SLV, 0:TQ],
                    in_=josb[0:SLV // 2, :])
            proj_unit_h(1, 0, ps=ps3[0])
            proj_unit_h(1, 1, ps=ps3[1])

    nc.compile()
    return nc


# ---------------------------------------------------------------- host glue
def _prep_in_maps(x, att_mask, w_kqv, b_kqv, w_proj, b_proj, masks):
    bf = bfloat16
    xt0 = x[0].T.astype(bf).reshape(KC, 128, T)               # [C, T]
    xT0 = np.ascontiguousarray(xt0[:, :, 0:TQ])
    xT0b = np.ascontiguousarray(xt0[:, :, TQ:2 * TQ])
    xT0c = np.ascontiguousarray(xt0[:, :, 2 * TQ:T])
    xt1 = x[1].T.astype(bf)
    xT1 = np.ascontiguousarray(xt1.reshape(KC, 128, T))
    wp_p = np.ascontiguousarray(
        w_proj.astype(bf).reshape(KC, 128, C).transpose(1, 0, 2)
        .reshape(128, KC * C))

    wk3 = w_kqv.reshape(C, H, 3, HD)
    bk3 = b_kqv.reshape(H, 3, HD)
    in_maps = []
    for core in range(NCORES):
        hA, hB = 2 * core, 2 * core + 1
        wk_c = np.concatenate(
            [np.concatenate([wk3[:, hA, s, :], wk3[:, hB, s, :]], axis=1)
             for s in range(3)], axis=1).astype(bf)           # [1024, 384]
        wk_p = np.ascontiguousarray(
            wk_c.reshape(KC, 128, 6 * HD).transpose(1, 0, 2)
            .reshape(128, KC * 6 * HD))
        bk_c = np.stack(
            [np.concatenate([bk3[hA, s], bk3[hB, s]])
             for s in range(3)], axis=1).astype(np.float32)   # [128, 3]
        im = {
            "xT0": xT0, "xT0b": xT0b, "xT0c": xT0c, "xT1": xT1,
            "wk": wk_p,
            "bk": np.ascontiguousarray(bk_c),
            "wp": wp_p,
            "bp": b_proj.reshape(1, C).astype(bf),
        }
        if masks is not None:
            im["mk"] = masks.astype(bf).reshape(-1, TQ)
        in_maps.append(im)
    return in_maps


def kernel(x, att_mask, w_kqv, b_kqv, w_proj, b_proj, n_head):
    from concourse.bass_utils import run_bass_kernel_spmd

    x = np.asarray(x, dtype=np.float32)
    att_mask = np.asarray(att_mask)
    w_kqv = np.asarray(w_kqv, dtype=np.float32)
    b_kqv = np.asarray(b_kqv, dtype=np.float32)
    w_proj = np.asarray(w_proj, dtype=np.float32)
    b_proj = np.asarray(b_proj, dtype=np.float32)
    n_head = int(n_head)
    assert x.shape == (B, T, C) and n_head == H

    sched, masks = _make_schedule(att_mask)
    key = _sched_key(sched, masks)
    if key not in _BUILD_CACHE:
        _BUILD_CACHE[key] = _build(sched, masks)
    nc = _BUILD_CACHE[key]

    in_maps = _prep_in_maps(x, att_mask, w_kqv, b_kqv, w_proj, b_proj, masks)
    res = run_bass_kernel_spmd(nc, in_maps, core_ids=list(range(NCORES)))

    out = np.empty((B, T, C), dtype=np.float32)
    for core in range(NCORES):
        arr = res.results[core]["out"].astype(np.float32)   # [NG, SLV, C]
        for g in range(3):
            c = 2 * g + (1 if core >= 4 else 0)
            b, Q = _chunk_bq(c)
            off = Q * TQ + (core % 4) * SLV
            out[b, off:off + SLV, :] = arr[g]
        # region 3: rows 0-63 = chunk 6 half-sliver, 64-127 = chunk 7
        HS = SLV // 2
        for u in range(2):
            b, Q = _chunk_bq(6 + u)
            off = Q * TQ + core * HS
            out[b, off:off + HS, :] = arr[3, u * HS:(u + 1) * HS]
    return out
